# revision 60
# baseline (speedup 1.0000x reference)
"""Multi-head self-attention (B=2, S=4096, D=512, H=8, Dh=64) on 8 TRN2 cores.

Sharding: core i handles batch b = i//4 and head-pair hp = i%4 (heads 2*hp,
2*hp+1).  Each core computes Q/K/V projections for its two heads, flash-style
attention (no-max softmax; scores range is +-9 so exp is safe), and a partial
out-projection.  Host sums the 4 partial outputs per batch and transposes back.

The kernel runs Scalar (exp) and Tensor in lockstep at ~1.09us/iteration:
ACT exp is (1024+352)cyc @1.2GHz ~= 1117ns per [128,1024] k-tile; the PE
wall per iteration (scores pair + 2 ctx matmuls + amortized projections) is
~1100ns.  Neither engine can shed work to the others (ctx streaming columns
are algorithmically fixed; exp is ACT-only at useful accuracy; a Schraudolph
int16-bitcast DVE exp was tried and is PE-capped -- code kept, gated off).
Organization so neither engine ever waits:
  - all matmul operands bfloat16: separate LDWEIGHTS pipeline into the PE
    background weight buffer, and lower power -> less HAM clock throttling
  - per (q-block, k-tile): the two heads' score matmuls are row-packed
    (PE rows 0-63 / 64-127, concurrent) into one [128,1024] PSUM tile, so
    ONE N=1024 exp covers both heads.  N=1024 is maximal: the ctx PSUM
    accumulator [65, QB] must fit one 2KB bank -> QB <= 512
  - ctx-lag: iteration i's ctx matmuls are emitted during iteration i+1, so
    their exp input is long done (no exp->ctx sem+fill exposure on the PE)
    and the next block's first ctx never stalls the in-order PE queue ahead
    of the exp-feeding scores
  - context accumulates per head in [65,512] PSUM via a V-with-ones-column
    stationary (row 64 = softmax denominator for free)
  - the out-projection of q-block i runs inside block i+1 split per head
    (K=64) against the UNNORMALIZED context, results scaled straight out of
    PSUM (DVE muls, GpSimd add) -- no staging copies; the 1/rowsum
    broadcast comes from a two-phase DMA-reshape normalize (phase B emitted
    early in block i+1 so the in-order DVE never parks on a DMA round-trip)
  - scores are software-pipelined one iteration ahead; Q/K/V projections
    race the attention loop in PE slack (V via v-major N=512 matmuls +
    PE transposes: N=128 matmuls are LDWEIGHTS-bound); X cols 0-511 arrive
    first on 4 parallel queues for the first block's projections, the rest
    in single 3-dim DMAs; 7 dummy ident matmuls ramp the PE p-state during
    the DMA wait
  - tail: the last block takes a short path (fused scale+cast of ctx once
    bc lands, then full-K out-projections)

Layouts (feature dim on partitions; every matmul contracts on partitions):
  xt  [512, S]  = X[b].T                       (bf16)
  wq/wk/wv [512, 128] = W[:, hp*128:(hp+1)*128] (bf16)
  wo  [128, 512] = Wo[hp*128:(hp+1)*128, :]     (bf16)
  yt  [512, S]  = partial (Y[b]).T              (fp32)

Known wall-clock structure (fast-clock trace, ~318us exec): head ~16us
(SPMD prologue ~7.5 + DMA/proj ramp), body ~274us (256 exps, ACT ~85% busy,
PE ~88%), tail ~29us (normalize chain ~8 + out-proj ~6 + DMA + two barrier
rounds ~8).  The chip has two DVFS states ~9% apart (ACTIVATE 1113 vs
1215ns); compare runs via the ACTIVATE median, not raw exec time.

TRN2 quirk: walrus legalizes only ONE sync wait on TPB compute instructions.
`_legalize_matmul_waits` moves extra waits onto injected single-wait no-ops.
"""

import sys
from contextlib import ExitStack

for _p in ("/opt/trn_rl_repo",):
    if _p not in sys.path:
        sys.path.insert(0, _p)

import numpy as np

import concourse.bass as bass
import concourse.tile as tile
from concourse import mybir
from concourse.bass_utils import run_bass_kernel_spmd
from concourse.masks import make_identity

F32 = mybir.dt.float32
BF16 = mybir.dt.bfloat16
I16 = mybir.dt.int16
MM_DT = BF16
# Schraudolph exp on the DVE: bf16 bits of e^x are approximated by
# int16(round(x*inv_scale*log2e*128 + (127*128 + C))); one tensor_scalar
# (fp32 PSUM in -> int16 SBUF out) bitcast to bf16 for the ctx matmul.
# Max rel err of the approx exp is ~3.3%; softmax normalization cancels
# most of it.  C tuned empirically (numpy scan; rounds-to-nearest).
SCH_BIAS = 127.0 * 128.0 - 5.5
EX_OFF_MIN_NK = 26   # enable the DVE exp offload only for full-size runs
D = 512          # model dim
DH = 64          # head dim
P = 128          # partitions
B = 2
H = 8
S_FULL = 4096
N_CORES = 8
NC_T = D // P    # 4 contraction tiles over model dim

LAST_RESULTS = None  # test harness reads exec_time_ns from here


def _emit(nc: bass.Bass, tc: "tile.TileContext", ctx: ExitStack, S: int):
    """Emit the per-core program. Parameterized by S for small-sim testing."""
    NK = S // P              # 128-row key tiles
    QB = 512                 # q-block (both heads processed per block)
    NQB = S // QB            # attention q-blocks
    inv_scale = 1.0 / np.sqrt(DH)
    sch_c1 = 128.0 * np.log2(np.e) * inv_scale

    def mm(out, lhsT, rhs, start=True, stop=True):
        return nc.tensor.matmul(out, lhsT, rhs, start=start, stop=stop)

    xt = nc.declare_dram_parameter("xt", [D, S], MM_DT, isOutput=False)
    wq = nc.declare_dram_parameter("wq", [D, P], MM_DT, isOutput=False)
    wk = nc.declare_dram_parameter("wk", [D, P], MM_DT, isOutput=False)
    wv = nc.declare_dram_parameter("wv", [D, P], MM_DT, isOutput=False)
    wo = nc.declare_dram_parameter("wo", [P, D], MM_DT, isOutput=False)
    yt = nc.declare_dram_parameter("yt", [D, S], F32, isOutput=True)

    const = ctx.enter_context(tc.tile_pool(name="const", bufs=1))

    # ---- weight DMA first (ONE dma each: a dma_start costs ~650ns of Sync
    # issue time regardless of size), then X in 8 pieces, first half first,
    # so the K projection for q-block 0 can start early ----
    w_sb = {}

    def dma_weight(name, ap):
        w4 = const.tile([P, NC_T * P], MM_DT, tag=f"{name}4", name=f"{name}4")
        wap = ap[:, :]
        base = wap.ap          # [[row_stride, 512], [col_stride, 128]]
        src = bass.AP(tensor=wap.tensor, offset=wap.offset,
                      ap=[[base[0][0], P], [base[0][0] * P, NC_T],
                          [base[1][0], P]])
        nc.sync.dma_start(out=w4.rearrange("p (c j) -> p c j", c=NC_T), in_=src)
        w_sb[name] = [w4[:, c * P:(c + 1) * P] for c in range(NC_T)]

    # one contiguous [P, 4*S] tile for X^T so each column-piece arrives in a
    # SINGLE 3-dim DMA covering all 4 contraction chunks (4 separate
    # dma_starts cost ~650ns of Sync issue time each)
    xt_all = const.tile([P, NC_T * S], MM_DT, tag="xt", name="xt_all")
    xt_v = xt_all.rearrange("p (c s) -> p c s", c=NC_T)

    def xt_c(c, lo, hi):
        return xt_all[:, c * S + lo:c * S + hi]

    def dma_xcols(lo, hi, split=False):
        if split:
            # head-latency path: 4 separate DMAs land on parallel queues,
            # so the first chunk arrives ~3x sooner than one big transfer
            for c in range(NC_T):
                nc.sync.dma_start(out=xt_v[:, c, lo:hi],
                                  in_=xt[c * P:(c + 1) * P, lo:hi])
            return
        base = xt[:, :].ap        # [[row_stride, 512], [col_stride, S]]
        src = bass.AP(tensor=xt, offset=xt[:, :].offset + lo * base[1][0],
                      ap=[[base[0][0], P], [base[0][0] * P, NC_T],
                          [base[1][0], hi - lo]])
        nc.sync.dma_start(out=xt_v[:, :, lo:hi], in_=src)

    # critical-path order: wk, then X cols 0-511 (all the first q-block's
    # projections need; chunk 0 lands first so the K projection starts
    # immediately), then wq/wv and the rest of X in 1024-col pieces
    dma_weight("wk", wk)
    dma_weight("wq", wq)
    dma_xcols(0, min(512, S), split=True)
    dma_weight("wv", wv)
    lo = 512
    while lo < S:
        hi = min(lo + 1024, S)
        dma_xcols(lo, hi)
        lo = hi
    wo_sb = const.tile([P, D], MM_DT, tag="wo")
    nc.sync.dma_start(out=wo_sb[:], in_=wo[:, :])

    # persistent intermediates
    qt_sb = const.tile([P, S], MM_DT, tag="qt")      # [2*64 d, S] stacked heads
    kt_sb = const.tile([P, S], MM_DT, tag="kt")
    # V with a ones column appended per k-tile: [128 k, NK*65]; col 64 == 1.0
    vones = [const.tile([P, NK * (DH + 1)], MM_DT, tag=f"vones{h}", name=f"vones{h}")
             for h in range(2)]
    konst = const.tile([P, NK, 1], F32, tag="konst")
    nc.vector.memset(konst[:], 1.0)
    for h in range(2):
        vv = vones[h].rearrange("p (k c) -> p k c", c=DH + 1)
        nc.vector.tensor_copy(vv[:, :, DH:DH + 1], konst[:])
    ctx_sb = const.tile([P, S], MM_DT, tag="ctx")    # context^T, stacked heads

    # PSUM banks: "s" 2 x [128,1024] (4) + "ctx" 2 x [65,512] (2) + "pp" 2 x
    # [128,512] (2) = 8.  Buffer *addresses* are assigned by autobufs
    # (interval packing); bufs= here are the per-tag caps.
    ps = ctx.enter_context(tc.tile_pool(name="ps", bufs=2, space="PSUM"))
    es = ctx.enter_context(tc.tile_pool(name="es", bufs=10))
    bcp = ctx.enter_context(tc.tile_pool(name="bcp", bufs=2))
    rtp = ctx.enter_context(tc.tile_pool(name="rtp", bufs=2))
    rdp = ctx.enter_context(tc.tile_pool(name="rdp", bufs=2, space="DRAM"))
    osb = ctx.enter_context(tc.tile_pool(name="osb", bufs=4))
    vtp = ctx.enter_context(tc.tile_pool(name="vtp", bufs=2))

    ident = const.tile([P, P], MM_DT, tag="ident")
    make_identity(nc, ident[:])

    # PE warm-up: ~3us of continuous dummy streaming during the initial DMA
    # wait ramps the Tensor engine to its top p-state before the first real
    # projection matmuls (cold matmuls run ~1.5x slower)
    nc.vector.memset(ctx_sb[:, 0:512], 0.0)
    warm = ps.tile([P, 512], F32, tag="pp", bufs=2, name="warm")
    for _ in range(7):
        mm(warm[:], ident[:], ctx_sb[:, 0:512])

    # ---- projections (tag "pp"; they fill PE slack under the ACT-bound
    # attention loop, racing ahead of their consumers) ----
    def proj_block(dst, wname, lo):
        """dst[:, lo:lo+512] = (W^T x)[:, lo:lo+512] over the 4 chunks."""
        pq = ps.tile([P, 512], F32, tag="pp", bufs=2, name=f"p{wname}")
        for c in range(NC_T):
            mm(pq[:], w_sb[wname][c][:], xt_c(c, lo, lo + 512),
               start=(c == 0), stop=(c == NC_T - 1))
        nc.vector.tensor_copy(dst[:, lo:lo + 512], pq[:])

    def proj_v_group(g, dma_t=False):
        """V for k-tiles 4g..4g+3, v-major N=512 matmuls (one stationary per
        chunk instead of per k-tile -- V at N=128 is LDWEIGHTS-bound), then
        transposed back to k-major for the vones layout.  dma_t=True uses
        the XBAR DMA transpose (Sync engine, idle mid-kernel) instead of PE
        transposes (PE is over budget while projections race q-block 0)."""
        sl = slice(g * 512, (g + 1) * 512)
        pv = ps.tile([P, 512], F32, tag="pp", bufs=2, name="pv")
        for c in range(NC_T):
            mm(pv[:], w_sb["wv"][c][:], xt_c(c, sl.start, sl.stop),
               start=(c == 0), stop=(c == NC_T - 1))
        vtmp = vtp.tile([P, 512], MM_DT, tag="vt", name="vtmp")
        nc.vector.tensor_copy(vtmp[:], pv[:])
        if dma_t:
            pt = vtp.tile([P, 512], MM_DT, tag="tst", name="tst")
            for j in range(4):
                nc.sync.dma_start_transpose(pt[:, j * P:(j + 1) * P],
                                            vtmp[:, j * P:(j + 1) * P])
        else:
            pt = ps.tile([P, 512], MM_DT, tag="pp", bufs=2, name="pt")
            for j in range(4):
                nc.tensor.transpose(pt[:, j * P:(j + 1) * P],
                                    vtmp[:, j * P:(j + 1) * P], ident[:])
        for j in range(4):
            k = 4 * g + j
            for h in range(2):
                nc.vector.tensor_copy(
                    vones[h][:, k * (DH + 1):k * (DH + 1) + DH],
                    pt[:, j * P + h * DH:j * P + (h + 1) * DH])

    # upfront on the critical path to the first exp: only K and Q for
    # q-block 0 (the first scores need nothing else); everything later
    # races the attention loop (Q blocks are emitted inside the loop, 4
    # k-iterations before their consumer)
    NSB = S // 512
    proj_block(kt_sb, "wk", 0)
    proj_block(qt_sb, "wq", 0)

    # ---- phase B + C interleaved ----
    # Per (qb, k): the two heads' score matmuls are row-packed -- h0 uses PE
    # rows 0-63 (base_partition 0), h1 rows 64-127 (base_partition 64) -- and
    # run CONCURRENTLY in disjoint row-groups, writing the two 512-col halves
    # (= two different banks) of one [128,1024] PSUM tile.  A single N=1024
    # exp then covers both heads, keeping the ACT cadence at (1024+352)/1.2
    # ~= 1147ns per k-tile while PE streaming is only ~650ns (fits under the
    # ACT cadence even when the power manager halves the PE clock).
    NW = 2 * QB // DH        # columns per partition in the [64, NW] reshape

    def normalize_a(ctx_pair):
        """Phase A of the rowsum normalize, emitted at the block boundary:
        pack both heads' [1,QB] rowsum rows side by side and DRAM-bounce
        them into a [64, NW] reshape (so the DVE reciprocal later costs
        ~NW columns, not 2*QB)."""
        r2 = rtp.tile([1, 2 * QB], F32, tag="rt", name="rt")
        for h in range(2):
            nc.vector.tensor_copy(r2[0:1, h * QB:(h + 1) * QB],
                                  ctx_pair[h][DH:DH + 1, :])
        rd1 = rdp.tile([1, 2 * QB], F32, tag="rd1", name="rd1")
        nc.sync.dma_start(out=rd1[:], in_=r2[0:1, :])
        el = list(rd1[0:1, :].ap)[1]           # [elem_stride, 2*QB]
        rd1_64 = bass.AP(tensor=rd1.tensor, offset=rd1.offset,
                         ap=[[el[0] * NW, DH], [el[0], NW]])
        r64 = rtp.tile([DH, NW], F32, tag="r64", name="r64")
        nc.sync.dma_start(out=r64[:], in_=rd1_64)
        return r64

    def normalize_b(r64):
        """Phase B, emitted a couple of iterations into the NEXT block so
        the in-order DVE queue never parks on the r64 DMA round-trip:
        reciprocal, bounce back to DRAM, and partition-stride-0 broadcast
        into two full-width [P, QB] bc tiles (NOT one [P, 2*QB] tile sliced
        in half: sliced reads break DVE free-dim coalescing and the combine
        muls run ~2.5x slower)."""
        rinv64 = rtp.tile([DH, NW], F32, tag="rinv64", name="rinv64")
        nc.vector.reciprocal(rinv64[:], r64[:])
        rd2 = rdp.tile([1, 2 * QB], F32, tag="rd2", name="rd2")
        el2 = list(rd2[0:1, :].ap)[1]
        rd2_64 = bass.AP(tensor=rd2.tensor, offset=rd2.offset,
                         ap=[[el2[0] * NW, DH], [el2[0], NW]])
        nc.sync.dma_start(out=rd2_64, in_=rinv64[:])
        bcs = []
        for h in range(2):
            bc = bcp.tile([P, QB], F32, tag=f"bc{h}", name=f"bc{h}")
            src = bass.AP(tensor=rd2.tensor, offset=rd2.offset + h * QB * el2[0],
                          ap=[[0, P], [el2[0], QB]])
            nc.sync.dma_start(out=bc[:], in_=src)
            bcs.append(bc)
        return bcs

    def out_mm(prev, idx):
        """Out-projection tile idx for the PREVIOUS q-block: two K=64
        row-packed matmuls against the UNNORMALIZED context.  The results
        stay in PSUM; out_mul scales them there directly (no staging copy
        -- the copies were ~44us of DVE time across the kernel)."""
        qsl, raws = prev["qsl"], []
        for h in range(2):
            hsl = slice(h * DH, (h + 1) * DH)
            o_ps = ps.tile([P, QB], F32, tag="pp", bufs=2, name="o_ps")
            mm(o_ps[:], wo_sb[hsl, idx * P:(idx + 1) * P], ctx_sb[hsl, qsl])
            raws.append(o_ps)
        prev["raw"][idx] = raws

    def out_mul(prev, idx):
        """m_h = o_ps_h * bc_h straight out of PSUM (frees the pp banks for
        the next out_mm pair).  GpSimd cannot read PSUM, so both muls ride
        the DVE; the SBUF-only add goes to GpSimd."""
        r0, r1 = prev["raw"][idx]
        m0 = osb.tile([P, QB], F32, tag="m0", bufs=2, name="m0")
        nc.vector.tensor_mul(m0[:], r0[:], prev["bc"][0][:])
        m1 = osb.tile([P, QB], F32, tag="m1", bufs=2, name="m1")
        nc.vector.tensor_mul(m1[:], r1[:], prev["bc"][1][:])
        prev["mul"][idx] = (m0, m1)

    def out_add(prev, idx):
        """o = m0 + m1 on the (otherwise idle) GpSimd engine, then DMA out."""
        m0, m1 = prev["mul"][idx]
        o_sb = osb.tile([P, QB], F32, tag="osb", bufs=2, name="o_sb")
        nc.gpsimd.tensor_add(o_sb[:], m0[:], m1[:])
        nc.sync.dma_start(out=yt[idx * P:(idx + 1) * P, prev["qsl"]],
                          in_=o_sb[:])

    def emit_scores(qb, k, i=0):
        qsl = slice(qb * QB, (qb + 1) * QB)
        s_pair = ps.tile([P, 2 * QB], F32, tag="s", name="s_pair")
        for h in range(2):
            hsl = slice(h * DH, (h + 1) * DH)
            mm(s_pair[:, h * QB:(h + 1) * QB],
               kt_sb[hsl, k * P:(k + 1) * P], qt_sb[hsl, qsl])
        return s_pair

    # Software-pipelined main loop: iteration i+1's score matmuls are
    # emitted (= prioritized) ahead of iteration i's ctx matmuls, so at a
    # q-block boundary the next block's first scores run before the old
    # block's final ctx pair and the exp stream never waits.
    iters = [(qb, k) for qb in range(NQB) for k in range(NK)]
    if NK >= 26:
        # out_mm waits only on ctx_sb, but out_mul needs bc (normalize DMA
        # chain, ~7us =~ 6 iterations): keep everything past k=7.  The DVE
        # muls sit on k%4==1 slots so they never queue in front of the
        # DVE-offloaded exps (k%4==2 slots).
        mm_pos = {8 + 4 * i: i for i in range(NC_T)}     # k = 8,12,16,20 (PE)
        ml_pos = {11 + 4 * i: i for i in range(NC_T)}    # k = 11,15,19,23 (DVE)
        ad_pos = {13 + 4 * i: i for i in range(NC_T)}    # k = 13,17,21,25 (GpSimd)
        # DVE exp offload abandoned: the PE per-iteration wall (~1104ns) sits
        # just under the ACT cadence (~1117ns), so offloading exps can save
        # at most ~3us of body and in practice loses more to the extra
        # latency of the DVE exp in the s-buffer double-buffer chain.
        ex_off = set()
    else:
        mm_pos = {i: i for i in range(NC_T)}
        ml_pos = {NC_T + i: i for i in range(NC_T)}
        ad_pos = dict(ml_pos)   # add right after its mul (NK is small)
        ex_off = set()
    assert not (set(mm_pos) & set(ml_pos)) and not (set(mm_pos) & set(ad_pos))
    assert not (ex_off & set(ml_pos))
    if NK < 26 and NK >= EX_OFF_MIN_NK:
        ex_off = {k for k in range(NK) if k % 4 == 2 and k != NK - 2}
    ctx_blocks = {}
    prev = None
    pend_ctx = None
    spair_next = emit_scores(0, 0, 0)
    # remaining projections, emitted after the first scores so they don't
    # delay the first exp; the scheduler runs them in PE slack.  K block g
    # and V group g are both consumed from score/ctx k-tile 4g on.
    if NSB > 1:
        proj_block(kt_sb, "wk", 512)
    proj_v_group(0)
    for g in range(1, NSB):
        if g >= 2:
            proj_block(kt_sb, "wk", g * 512)
        proj_v_group(g)
    for i, (qb, k) in enumerate(iters):
        qsl = slice(qb * QB, (qb + 1) * QB)
        if k == 0:
            ctx_blocks[qb] = [ps.tile([DH + 1, QB], F32, tag="ctx", bufs=2,
                                      name=f"ctx_ps{h}") for h in range(2)]
        s_pair, ctx_h = spair_next, ctx_blocks[qb]
        if i + 1 < len(iters):
            spair_next = emit_scores(*iters[i + 1], i + 1)
        if k in ex_off:
            # ~1/4 of the exps ride the DVE as a one-instruction Schraudolph
            # (f32 affine -> int16 convert, bitcast to bf16), freeing the
            # Scalar engine -- the kernel's bottleneck
            ei = es.tile([P, 2 * QB], I16, tag="ei", bufs=4, name="e_i16")
            nc.vector.tensor_scalar(ei[:], s_pair[:], sch_c1, SCH_BIAS,
                                    mybir.AluOpType.mult, mybir.AluOpType.add)
            e_aps = [ei[:, h * QB:(h + 1) * QB].bitcast(BF16)
                     for h in range(2)]
        else:
            e_pair = es.tile([P, 2 * QB], MM_DT, tag="e", name="e_pair")
            nc.scalar.activation(e_pair[:], s_pair[:],
                                 mybir.ActivationFunctionType.Exp,
                                 scale=inv_scale)
            e_aps = [e_pair[:, h * QB:(h + 1) * QB] for h in range(2)]

        # ctx-lag: iteration i's ctx matmuls are EMITTED during iteration
        # i+1, so their e input is long done and the PE never pays the
        # exp->ctx sem+pipeline-fill exposure; at a block boundary this also
        # keeps the next block's first ctx (which WARs on the normalize /
        # staging reads of the old ctx banks) from stalling the in-order PE
        # queue ahead of the exp-feeding scores.
        def emit_ctx(kk, ctx_hh, aps):
            for h in range(2):
                vo = vones[h][:, kk * (DH + 1):(kk + 1) * (DH + 1)]
                mm(ctx_hh[h][:], vo, aps[h],
                   start=(kk == 0), stop=(kk == NK - 1))
        if pend_ctx is not None:
            emit_ctx(*pend_ctx)
        pend_ctx = (k, ctx_h, e_aps)
        if k == NK - 1:
            emit_ctx(*pend_ctx)       # flush before the block-end chunk
            pend_ctx = None
        # previous q-block's out-projection, spread over this block's
        # PE/DVE slack: matmuls (vs unnormalized ctx -- no wait on the
        # normalize chain) every 3rd iteration, scale-combines after.
        # Spacing matters: bursting these backs up the DVE with o_raw
        # copies and the in-order PE queue stalls behind an out_mm
        # waiting on it, starving the exp stream.
        if prev is not None:
            if k in mm_pos:
                out_mm(prev, mm_pos[k])
            if k in ml_pos:
                out_mul(prev, ml_pos[k])
            if k in ad_pos:
                out_add(prev, ad_pos[k])
        # next q-block's Q projection, 4 iterations before its consumer
        # (the score-ahead emission at k == NK-1)
        if k == NK - 8 and qb + 1 < NQB:
            proj_block(qt_sb, "wq", (qb + 1) * 512)
        if k == 2 and prev is not None and prev["bc"] is None:
            prev["bc"] = normalize_b(prev["r64"])
        if k == NK - 1:
            last = (qb == NQB - 1)
            # normalize phase A first: its tiny r2 copies unblock the DMA
            # bounce before the boundary staging work
            r64 = normalize_a(ctx_blocks[qb])
            if not last:
                # stage the unnormalized context out of PSUM (bf16); the
                # 1/rowsum scales are applied by out_mul later
                for h in range(2):
                    nc.vector.tensor_copy(ctx_sb[h * DH:(h + 1) * DH, qsl],
                                          ctx_h[h][:DH, :])
            prev = {"qsl": qsl, "raw": [None] * NC_T, "mul": [None] * NC_T,
                    "bc": None, "r64": r64}
    # final q-block: nothing left to pipeline against, so take the short
    # path -- fused scale+cast of the context out of PSUM once bc lands
    # (the DVE is otherwise idle, so the normalize chain runs unclogged),
    # then full-K out-projections
    if prev["bc"] is None:
        prev["bc"] = normalize_b(prev["r64"])
    qsl = prev["qsl"]
    for h in range(2):
        nc.vector.tensor_mul(ctx_sb[h * DH:(h + 1) * DH, qsl],
                             ctx_blocks[NQB - 1][h][:DH, :],
                             prev["bc"][h][0:DH, :])
    for idx in range(NC_T):
        o_ps = ps.tile([P, QB], F32, tag="pp", bufs=2, name="o_ps")
        mm(o_ps[:], wo_sb[:, idx * P:(idx + 1) * P], ctx_sb[:, qsl])
        o_sb = osb.tile([P, QB], F32, tag="osb", bufs=2, name="o_sb")
        nc.vector.tensor_copy(o_sb[:], o_ps[:])
        nc.sync.dma_start(out=yt[idx * P:(idx + 1) * P, qsl], in_=o_sb[:])


_TPB_ENGINES = {mybir.EngineType.PE, mybir.EngineType.Activation,
                mybir.EngineType.DVE, mybir.EngineType.Pool}


def _legalize_matmul_waits(nc: bass.Bass) -> int:
    """Walrus encodes only ONE sync wait on TPB compute instructions (seen on
    Matmult and TensorCopy).  Move extra waits onto injected same-engine
    no-ops (one wait each) placed immediately before the instruction in its
    block: same semantics, legal encoding."""
    n_fixed = 0
    for f in nc.m.functions:
        for bb in f.blocks:
            out = []
            changed = False
            for ins in bb.instructions:
                si = ins.sync_info
                if (getattr(ins, "engine", None) is not None
                        and si is not None and len(si.on_wait) > 1):
                    for idx, w in enumerate(si.on_wait[:-1]):
                        nop = mybir.InstNoOp(name=f"{ins.name}-lgw{idx}",
                                             ins=[], outs=[])
                        nop.engine = ins.engine
                        nop.sync_info = mybir.SyncInfo(on_wait=[w], on_update=[])
                        out.append(nop)
                    ins.sync_info = mybir.SyncInfo(on_wait=[si.on_wait[-1]],
                                                   on_update=si.on_update)
                    n_fixed += 1
                    changed = True
                out.append(ins)
            if changed:
                bb.instructions = out
    return n_fixed


def build(S: int = S_FULL, legalize: bool = False) -> bass.Bass:
    nc = bass.Bass()
    with ExitStack() as ctx:
        ctx.enter_context(nc.allow_low_precision(
            reason="bf16 matmul operands / intermediates"))
        tc = ctx.enter_context(tile.TileContext(nc))
        _emit(nc, tc, ctx, S)
    if legalize:
        # only for the walrus/hardware path; CoreSim wants updates on every
        # instruction and doesn't enforce the 1-wait Matmult limit
        _legalize_matmul_waits(nc)
    return nc


_NC_CACHE = {}


def _get_nc(S: int) -> bass.Bass:
    if S not in _NC_CACHE:
        _NC_CACHE[S] = build(S, legalize=True)
    return _NC_CACHE[S]


def _bf16(a):
    import ml_dtypes
    return np.ascontiguousarray(np.asarray(a, dtype=np.float32)).astype(
        ml_dtypes.bfloat16)


def make_in_maps(X, Wq, Wk, Wv, Wo):
    X = np.asarray(X, dtype=np.float32)
    xts = [_bf16(X[b].T) for b in range(B)]
    in_maps = []
    for i in range(N_CORES):
        b, hp = divmod(i, 4)  # 4 head-pairs per batch
        csl = slice(hp * P, (hp + 1) * P)
        in_maps.append({
            "xt": xts[b],
            "wq": _bf16(Wq[:, csl]),
            "wk": _bf16(Wk[:, csl]),
            "wv": _bf16(Wv[:, csl]),
            "wo": _bf16(Wo[csl, :]),
        })
    return in_maps


def kernel(X, Wq, Wk, Wv, Wo, _trace=False):
    global LAST_RESULTS
    X = np.asarray(X, dtype=np.float32)
    S = X.shape[1]
    nc = _get_nc(S)
    in_maps = make_in_maps(X, np.asarray(Wq, np.float32), np.asarray(Wk, np.float32),
                           np.asarray(Wv, np.float32), np.asarray(Wo, np.float32))
    res = run_bass_kernel_spmd(nc, in_maps, list(range(N_CORES)), trace=_trace)
    LAST_RESULTS = res
    Y = np.zeros((B, S, D), dtype=np.float32)
    for i in range(N_CORES):
        Y[i // 4] += res.results[i]["yt"].T
    return Y



# revision 62
# speedup vs baseline: 1.0212x; 1.0212x over previous
"""Multi-head self-attention (B=2, S=4096, D=512, H=8, Dh=64) on 8 TRN2 cores.

Sharding: core i handles batch b = i//4 and head-pair hp = i%4 (heads 2*hp,
2*hp+1).  Each core computes Q/K/V projections for its two heads, flash-style
attention (no-max softmax; scores range is +-9 so exp is safe), and a partial
out-projection.  Host sums the 4 partial outputs per batch and transposes back.

The kernel runs Scalar (exp) and Tensor in lockstep at ~1.09us/iteration:
ACT exp is (1024+352)cyc @1.2GHz ~= 1117ns per [128,1024] k-tile; the PE
wall per iteration (scores pair + 2 ctx matmuls + amortized projections) is
~1100ns.  Neither engine can shed work to the others (ctx streaming columns
are algorithmically fixed; exp is ACT-only at useful accuracy; a Schraudolph
int16-bitcast DVE exp was tried and is PE-capped -- code kept, gated off).
Organization so neither engine ever waits:
  - all matmul operands bfloat16: separate LDWEIGHTS pipeline into the PE
    background weight buffer, and lower power -> less HAM clock throttling
  - per (q-block, k-tile): the two heads' score matmuls are row-packed
    (PE rows 0-63 / 64-127, concurrent) into one [128,1024] PSUM tile, so
    ONE N=1024 exp covers both heads.  N=1024 is maximal: the ctx PSUM
    accumulator [65, QB] must fit one 2KB bank -> QB <= 512
  - ctx-lag: iteration i's ctx matmuls are emitted during iteration i+1, so
    their exp input is long done (no exp->ctx sem+fill exposure on the PE)
    and the next block's first ctx never stalls the in-order PE queue ahead
    of the exp-feeding scores
  - context accumulates per head in [65,512] PSUM via a V-with-ones-column
    stationary (row 64 = softmax denominator for free)
  - the out-projection of q-block i runs inside block i+1 split per head
    (K=64) against the UNNORMALIZED context, results scaled straight out of
    PSUM (DVE muls, GpSimd add) -- no staging copies; the 1/rowsum
    broadcast comes from a two-phase DMA-reshape normalize (phase B emitted
    early in block i+1 so the in-order DVE never parks on a DMA round-trip)
  - scores are software-pipelined one iteration ahead; Q/K/V projections
    race the attention loop in PE slack (V via v-major N=512 matmuls +
    PE transposes: N=128 matmuls are LDWEIGHTS-bound); X cols 0-511 arrive
    first on 4 parallel queues for the first block's projections, the rest
    in single 3-dim DMAs; 7 dummy ident matmuls ramp the PE p-state during
    the DMA wait
  - tail: the last block takes a short path (fused scale+cast of ctx once
    bc lands, then full-K out-projections)

Layouts (feature dim on partitions; every matmul contracts on partitions):
  xt  [512, S]  = X[b].T                       (bf16)
  wq/wk/wv [512, 128] = W[:, hp*128:(hp+1)*128] (bf16)
  wo  [128, 512] = Wo[hp*128:(hp+1)*128, :]     (bf16)
  yt  [512, S]  = partial (Y[b]).T              (fp32)

Known wall-clock structure (fast-clock trace, ~318us exec): head ~16us
(SPMD prologue ~7.5 + DMA/proj ramp), body ~274us (256 exps, ACT ~85% busy,
PE ~88%), tail ~29us (normalize chain ~8 + out-proj ~6 + DMA + two barrier
rounds ~8).  The chip has two DVFS states ~9% apart (ACTIVATE 1113 vs
1215ns); compare runs via the ACTIVATE median, not raw exec time.

TRN2 quirk: walrus legalizes only ONE sync wait on TPB compute instructions.
`_legalize_matmul_waits` moves extra waits onto injected single-wait no-ops.
"""

import sys
from contextlib import ExitStack

for _p in ("/opt/trn_rl_repo",):
    if _p not in sys.path:
        sys.path.insert(0, _p)

import numpy as np

import concourse.bass as bass
import concourse.tile as tile
from concourse import mybir
from concourse.bass_utils import run_bass_kernel_spmd
from concourse.masks import make_identity

F32 = mybir.dt.float32
BF16 = mybir.dt.bfloat16
I16 = mybir.dt.int16
MM_DT = BF16
# Schraudolph exp on the DVE: bf16 bits of e^x are approximated by
# int16(round(x*inv_scale*log2e*128 + (127*128 + C))); one tensor_scalar
# (fp32 PSUM in -> int16 SBUF out) bitcast to bf16 for the ctx matmul.
# Max rel err of the approx exp is ~3.3%; softmax normalization cancels
# most of it.  C tuned empirically (numpy scan; rounds-to-nearest).
SCH_BIAS = 127.0 * 128.0 - 5.5
EX_OFF_MIN_NK = 26   # enable the DVE exp offload only for full-size runs
D = 512          # model dim
DH = 64          # head dim
P = 128          # partitions
B = 2
H = 8
S_FULL = 4096
N_CORES = 8
NC_T = D // P    # 4 contraction tiles over model dim

LAST_RESULTS = None  # test harness reads exec_time_ns from here


def _emit(nc: bass.Bass, tc: "tile.TileContext", ctx: ExitStack, S: int):
    """Emit the per-core program. Parameterized by S for small-sim testing."""
    NK = S // P              # 128-row key tiles
    QB = 512                 # q-block (both heads processed per block)
    NQB = S // QB            # attention q-blocks
    inv_scale = 1.0 / np.sqrt(DH)
    sch_c1 = 128.0 * np.log2(np.e) * inv_scale

    def mm(out, lhsT, rhs, start=True, stop=True):
        return nc.tensor.matmul(out, lhsT, rhs, start=start, stop=stop)

    xt = nc.declare_dram_parameter("xt", [D, S], MM_DT, isOutput=False)
    wq = nc.declare_dram_parameter("wq", [D, P], MM_DT, isOutput=False)
    wk = nc.declare_dram_parameter("wk", [D, P], MM_DT, isOutput=False)
    wv = nc.declare_dram_parameter("wv", [D, P], MM_DT, isOutput=False)
    wo = nc.declare_dram_parameter("wo", [P, D], MM_DT, isOutput=False)
    yt = nc.declare_dram_parameter("yt", [D, S], F32, isOutput=True)

    const = ctx.enter_context(tc.tile_pool(name="const", bufs=1))

    # ---- weight DMA first (ONE dma each: a dma_start costs ~650ns of Sync
    # issue time regardless of size), then X in 8 pieces, first half first,
    # so the K projection for q-block 0 can start early ----
    w_sb = {}

    def dma_weight(name, ap):
        w4 = const.tile([P, NC_T * P], MM_DT, tag=f"{name}4", name=f"{name}4")
        wap = ap[:, :]
        base = wap.ap          # [[row_stride, 512], [col_stride, 128]]
        src = bass.AP(tensor=wap.tensor, offset=wap.offset,
                      ap=[[base[0][0], P], [base[0][0] * P, NC_T],
                          [base[1][0], P]])
        nc.sync.dma_start(out=w4.rearrange("p (c j) -> p c j", c=NC_T), in_=src)
        w_sb[name] = [w4[:, c * P:(c + 1) * P] for c in range(NC_T)]

    # one contiguous [P, 4*S] tile for X^T so each column-piece arrives in a
    # SINGLE 3-dim DMA covering all 4 contraction chunks (4 separate
    # dma_starts cost ~650ns of Sync issue time each)
    xt_all = const.tile([P, NC_T * S], MM_DT, tag="xt", name="xt_all")
    xt_v = xt_all.rearrange("p (c s) -> p c s", c=NC_T)

    def xt_c(c, lo, hi):
        return xt_all[:, c * S + lo:c * S + hi]

    def dma_xcols(lo, hi, split=False):
        if split:
            # head-latency path: 4 separate DMAs land on parallel queues,
            # so the first chunk arrives ~3x sooner than one big transfer
            for c in range(NC_T):
                nc.sync.dma_start(out=xt_v[:, c, lo:hi],
                                  in_=xt[c * P:(c + 1) * P, lo:hi])
            return
        base = xt[:, :].ap        # [[row_stride, 512], [col_stride, S]]
        src = bass.AP(tensor=xt, offset=xt[:, :].offset + lo * base[1][0],
                      ap=[[base[0][0], P], [base[0][0] * P, NC_T],
                          [base[1][0], hi - lo]])
        nc.sync.dma_start(out=xt_v[:, :, lo:hi], in_=src)

    # critical-path order: wk, then X cols 0-511 (all the first q-block's
    # projections need; chunk 0 lands first so the K projection starts
    # immediately), then wq/wv and the rest of X in 1024-col pieces
    dma_weight("wk", wk)
    dma_weight("wq", wq)
    dma_xcols(0, min(512, S), split=True)
    dma_weight("wv", wv)
    lo = 512
    while lo < S:
        hi = min(lo + 1024, S)
        dma_xcols(lo, hi)
        lo = hi
    wo_sb = const.tile([P, D], MM_DT, tag="wo")
    nc.sync.dma_start(out=wo_sb[:], in_=wo[:, :])

    # persistent intermediates
    qt_sb = const.tile([P, S], MM_DT, tag="qt")      # [2*64 d, S] stacked heads
    kt_sb = const.tile([P, S], MM_DT, tag="kt")
    # V with a ones column appended per k-tile: [128 k, NK*65]; col 64 == 1.0
    vones = [const.tile([P, NK * (DH + 1)], MM_DT, tag=f"vones{h}", name=f"vones{h}")
             for h in range(2)]
    konst = const.tile([P, NK, 1], F32, tag="konst")
    nc.vector.memset(konst[:], 1.0)
    for h in range(2):
        vv = vones[h].rearrange("p (k c) -> p k c", c=DH + 1)
        nc.vector.tensor_copy(vv[:, :, DH:DH + 1], konst[:])
    ctx_sb = const.tile([P, S], MM_DT, tag="ctx")    # context^T, stacked heads

    # PSUM banks: "s" 2 x [128,1024] (4) + "ctx" 2 x [65,512] (2) + "pp" 2 x
    # [128,512] (2) = 8.  Buffer *addresses* are assigned by autobufs
    # (interval packing); bufs= here are the per-tag caps.
    ps = ctx.enter_context(tc.tile_pool(name="ps", bufs=2, space="PSUM"))
    es = ctx.enter_context(tc.tile_pool(name="es", bufs=10))
    bcp = ctx.enter_context(tc.tile_pool(name="bcp", bufs=2))
    rtp = ctx.enter_context(tc.tile_pool(name="rtp", bufs=2))
    rdp = ctx.enter_context(tc.tile_pool(name="rdp", bufs=2, space="DRAM"))
    osb = ctx.enter_context(tc.tile_pool(name="osb", bufs=4))
    vtp = ctx.enter_context(tc.tile_pool(name="vtp", bufs=2))

    ident = const.tile([P, P], MM_DT, tag="ident")
    make_identity(nc, ident[:])

    # PE warm-up: ~3us of continuous dummy streaming during the initial DMA
    # wait ramps the Tensor engine to its top p-state before the first real
    # projection matmuls (cold matmuls run ~1.5x slower)
    nc.vector.memset(ctx_sb[:, 0:512], 0.0)
    warm = ps.tile([P, 512], F32, tag="pp", bufs=2, name="warm")
    for _ in range(7):
        mm(warm[:], ident[:], ctx_sb[:, 0:512])

    # ---- projections (tag "pp"; they fill PE slack under the ACT-bound
    # attention loop, racing ahead of their consumers) ----
    def proj_block(dst, wname, lo):
        """dst[:, lo:lo+512] = (W^T x)[:, lo:lo+512] over the 4 chunks."""
        pq = ps.tile([P, 512], F32, tag="pp", bufs=2, name=f"p{wname}")
        for c in range(NC_T):
            mm(pq[:], w_sb[wname][c][:], xt_c(c, lo, lo + 512),
               start=(c == 0), stop=(c == NC_T - 1))
        nc.vector.tensor_copy(dst[:, lo:lo + 512], pq[:])

    def proj_v_group(g, dma_t=False):
        """V for k-tiles 4g..4g+3, v-major N=512 matmuls (one stationary per
        chunk instead of per k-tile -- V at N=128 is LDWEIGHTS-bound), then
        transposed back to k-major for the vones layout.  dma_t=True uses
        the XBAR DMA transpose (Sync engine, idle mid-kernel) instead of PE
        transposes (PE is over budget while projections race q-block 0)."""
        sl = slice(g * 512, (g + 1) * 512)
        pv = ps.tile([P, 512], F32, tag="pp", bufs=2, name="pv")
        for c in range(NC_T):
            mm(pv[:], w_sb["wv"][c][:], xt_c(c, sl.start, sl.stop),
               start=(c == 0), stop=(c == NC_T - 1))
        vtmp = vtp.tile([P, 512], MM_DT, tag="vt", name="vtmp")
        nc.vector.tensor_copy(vtmp[:], pv[:])
        if dma_t:
            pt = vtp.tile([P, 512], MM_DT, tag="tst", name="tst")
            for j in range(4):
                nc.sync.dma_start_transpose(pt[:, j * P:(j + 1) * P],
                                            vtmp[:, j * P:(j + 1) * P])
        else:
            pt = ps.tile([P, 512], MM_DT, tag="pp", bufs=2, name="pt")
            for j in range(4):
                nc.tensor.transpose(pt[:, j * P:(j + 1) * P],
                                    vtmp[:, j * P:(j + 1) * P], ident[:])
        for j in range(4):
            k = 4 * g + j
            for h in range(2):
                nc.vector.tensor_copy(
                    vones[h][:, k * (DH + 1):k * (DH + 1) + DH],
                    pt[:, j * P + h * DH:j * P + (h + 1) * DH])

    # upfront on the critical path to the first exp: only K and Q for
    # q-block 0 (the first scores need nothing else); everything later
    # races the attention loop (Q blocks are emitted inside the loop, 4
    # k-iterations before their consumer)
    NSB = S // 512
    # first K and Q projections interleaved per chunk on two PSUM
    # accumulators, so each chunk's Q matmul runs right behind its K matmul
    # as the X chunks land (instead of Q waiting for all of K); the kt cast
    # rides the Scalar engine, which is idle until the first exp
    pk0 = ps.tile([P, 512], F32, tag="pp", bufs=2, name="pk0")
    pq0 = ps.tile([P, 512], F32, tag="pp", bufs=2, name="pq0")
    for c in range(NC_T):
        mm(pk0[:], w_sb["wk"][c][:], xt_c(c, 0, 512),
           start=(c == 0), stop=(c == NC_T - 1))
        mm(pq0[:], w_sb["wq"][c][:], xt_c(c, 0, 512),
           start=(c == 0), stop=(c == NC_T - 1))
    nc.scalar.copy(kt_sb[:, 0:512], pk0[:])
    nc.vector.tensor_copy(qt_sb[:, 0:512], pq0[:])

    # ---- phase B + C interleaved ----
    # Per (qb, k): the two heads' score matmuls are row-packed -- h0 uses PE
    # rows 0-63 (base_partition 0), h1 rows 64-127 (base_partition 64) -- and
    # run CONCURRENTLY in disjoint row-groups, writing the two 512-col halves
    # (= two different banks) of one [128,1024] PSUM tile.  A single N=1024
    # exp then covers both heads, keeping the ACT cadence at (1024+352)/1.2
    # ~= 1147ns per k-tile while PE streaming is only ~650ns (fits under the
    # ACT cadence even when the power manager halves the PE clock).
    NW = 2 * QB // DH        # columns per partition in the [64, NW] reshape

    def normalize_a(ctx_pair):
        """Phase A of the rowsum normalize, emitted at the block boundary:
        pack both heads' [1,QB] rowsum rows side by side and scatter them
        into a [64, NW] reshape (so the DVE reciprocal later costs ~NW
        columns, not 2*QB) with a single SBUF->SBUF DMA -- the flattened
        element orders of a [1, 2*QB] row and a [64, NW] tile coincide, so
        no DRAM bounce is needed for this leg."""
        r2 = rtp.tile([1, 2 * QB], F32, tag="rt", name="rt")
        for h in range(2):
            nc.vector.tensor_copy(r2[0:1, h * QB:(h + 1) * QB],
                                  ctx_pair[h][DH:DH + 1, :])
        r64 = rtp.tile([DH, NW], F32, tag="r64", name="r64")
        nc.sync.dma_start(out=r64[:], in_=r2[0:1, :])
        return r64

    def normalize_b(r64):
        """Phase B, emitted a couple of iterations into the NEXT block so
        the in-order DVE queue never parks on the r64 DMA round-trip:
        reciprocal, bounce back to DRAM, and partition-stride-0 broadcast
        into two full-width [P, QB] bc tiles (NOT one [P, 2*QB] tile sliced
        in half: sliced reads break DVE free-dim coalescing and the combine
        muls run ~2.5x slower)."""
        rinv64 = rtp.tile([DH, NW], F32, tag="rinv64", name="rinv64")
        nc.vector.reciprocal(rinv64[:], r64[:])
        rd2 = rdp.tile([1, 2 * QB], F32, tag="rd2", name="rd2")
        el2 = list(rd2[0:1, :].ap)[1]
        rd2_64 = bass.AP(tensor=rd2.tensor, offset=rd2.offset,
                         ap=[[el2[0] * NW, DH], [el2[0], NW]])
        nc.sync.dma_start(out=rd2_64, in_=rinv64[:])
        bcs = []
        for h in range(2):
            bc = bcp.tile([P, QB], F32, tag=f"bc{h}", name=f"bc{h}")
            src = bass.AP(tensor=rd2.tensor, offset=rd2.offset + h * QB * el2[0],
                          ap=[[0, P], [el2[0], QB]])
            nc.sync.dma_start(out=bc[:], in_=src)
            bcs.append(bc)
        return bcs

    def out_mm(prev, idx):
        """Out-projection tile idx for the PREVIOUS q-block: two K=64
        row-packed matmuls against the UNNORMALIZED context.  The results
        stay in PSUM; out_mul scales them there directly (no staging copy
        -- the copies were ~44us of DVE time across the kernel)."""
        qsl, raws = prev["qsl"], []
        for h in range(2):
            hsl = slice(h * DH, (h + 1) * DH)
            o_ps = ps.tile([P, QB], F32, tag="pp", bufs=2, name="o_ps")
            mm(o_ps[:], wo_sb[hsl, idx * P:(idx + 1) * P], ctx_sb[hsl, qsl])
            raws.append(o_ps)
        prev["raw"][idx] = raws

    def out_mul(prev, idx):
        """m_h = o_ps_h * bc_h straight out of PSUM (frees the pp banks for
        the next out_mm pair).  GpSimd cannot read PSUM, so both muls ride
        the DVE; the SBUF-only add goes to GpSimd."""
        r0, r1 = prev["raw"][idx]
        m0 = osb.tile([P, QB], F32, tag="m0", bufs=2, name="m0")
        nc.vector.tensor_mul(m0[:], r0[:], prev["bc"][0][:])
        m1 = osb.tile([P, QB], F32, tag="m1", bufs=2, name="m1")
        nc.vector.tensor_mul(m1[:], r1[:], prev["bc"][1][:])
        prev["mul"][idx] = (m0, m1)

    def out_add(prev, idx):
        """o = m0 + m1 on the (otherwise idle) GpSimd engine, then DMA out."""
        m0, m1 = prev["mul"][idx]
        o_sb = osb.tile([P, QB], F32, tag="osb", bufs=2, name="o_sb")
        nc.gpsimd.tensor_add(o_sb[:], m0[:], m1[:])
        nc.sync.dma_start(out=yt[idx * P:(idx + 1) * P, prev["qsl"]],
                          in_=o_sb[:])

    def emit_scores(qb, k, i=0):
        qsl = slice(qb * QB, (qb + 1) * QB)
        s_pair = ps.tile([P, 2 * QB], F32, tag="s", name="s_pair")
        for h in range(2):
            hsl = slice(h * DH, (h + 1) * DH)
            mm(s_pair[:, h * QB:(h + 1) * QB],
               kt_sb[hsl, k * P:(k + 1) * P], qt_sb[hsl, qsl])
        return s_pair

    # Software-pipelined main loop: iteration i+1's score matmuls are
    # emitted (= prioritized) ahead of iteration i's ctx matmuls, so at a
    # q-block boundary the next block's first scores run before the old
    # block's final ctx pair and the exp stream never waits.
    iters = [(qb, k) for qb in range(NQB) for k in range(NK)]
    if NK >= 26:
        # out_mm waits only on ctx_sb, but out_mul needs bc (normalize DMA
        # chain, ~7us =~ 6 iterations): keep everything past k=7.  The DVE
        # muls sit on k%4==1 slots so they never queue in front of the
        # DVE-offloaded exps (k%4==2 slots).
        mm_pos = {8 + 4 * i: i for i in range(NC_T)}     # k = 8,12,16,20 (PE)
        ml_pos = {11 + 4 * i: i for i in range(NC_T)}    # k = 11,15,19,23 (DVE)
        ad_pos = {13 + 4 * i: i for i in range(NC_T)}    # k = 13,17,21,25 (GpSimd)
        # DVE exp offload abandoned: the PE per-iteration wall (~1104ns) sits
        # just under the ACT cadence (~1117ns), so offloading exps can save
        # at most ~3us of body and in practice loses more to the extra
        # latency of the DVE exp in the s-buffer double-buffer chain.
        ex_off = set()
    else:
        mm_pos = {i: i for i in range(NC_T)}
        ml_pos = {NC_T + i: i for i in range(NC_T)}
        ad_pos = dict(ml_pos)   # add right after its mul (NK is small)
        ex_off = set()
    assert not (set(mm_pos) & set(ml_pos)) and not (set(mm_pos) & set(ad_pos))
    assert not (ex_off & set(ml_pos))
    if NK < 26 and NK >= EX_OFF_MIN_NK:
        ex_off = {k for k in range(NK) if k % 4 == 2 and k != NK - 2}
    ctx_blocks = {}
    prev = None
    pend_ctx = None
    spair_next = emit_scores(0, 0, 0)
    # remaining projections, emitted after the first scores so they don't
    # delay the first exp; the scheduler runs them in PE slack.  K block g
    # and V group g are both consumed from score/ctx k-tile 4g on.
    if NSB > 1:
        proj_block(kt_sb, "wk", 512)
    proj_v_group(0)
    for g in range(1, NSB):
        if g >= 2:
            proj_block(kt_sb, "wk", g * 512)
        proj_v_group(g)
    for i, (qb, k) in enumerate(iters):
        qsl = slice(qb * QB, (qb + 1) * QB)
        if k == 0:
            ctx_blocks[qb] = [ps.tile([DH + 1, QB], F32, tag="ctx", bufs=2,
                                      name=f"ctx_ps{h}") for h in range(2)]
        s_pair, ctx_h = spair_next, ctx_blocks[qb]
        if i + 1 < len(iters):
            spair_next = emit_scores(*iters[i + 1], i + 1)
        if k in ex_off:
            # ~1/4 of the exps ride the DVE as a one-instruction Schraudolph
            # (f32 affine -> int16 convert, bitcast to bf16), freeing the
            # Scalar engine -- the kernel's bottleneck
            ei = es.tile([P, 2 * QB], I16, tag="ei", bufs=4, name="e_i16")
            nc.vector.tensor_scalar(ei[:], s_pair[:], sch_c1, SCH_BIAS,
                                    mybir.AluOpType.mult, mybir.AluOpType.add)
            e_aps = [ei[:, h * QB:(h + 1) * QB].bitcast(BF16)
                     for h in range(2)]
        else:
            e_pair = es.tile([P, 2 * QB], MM_DT, tag="e", name="e_pair")
            nc.scalar.activation(e_pair[:], s_pair[:],
                                 mybir.ActivationFunctionType.Exp,
                                 scale=inv_scale)
            e_aps = [e_pair[:, h * QB:(h + 1) * QB] for h in range(2)]

        # ctx-lag: iteration i's ctx matmuls are EMITTED during iteration
        # i+1, so their e input is long done and the PE never pays the
        # exp->ctx sem+pipeline-fill exposure; at a block boundary this also
        # keeps the next block's first ctx (which WARs on the normalize /
        # staging reads of the old ctx banks) from stalling the in-order PE
        # queue ahead of the exp-feeding scores.
        def emit_ctx(kk, ctx_hh, aps):
            for h in range(2):
                vo = vones[h][:, kk * (DH + 1):(kk + 1) * (DH + 1)]
                mm(ctx_hh[h][:], vo, aps[h],
                   start=(kk == 0), stop=(kk == NK - 1))
        if pend_ctx is not None:
            emit_ctx(*pend_ctx)
        pend_ctx = (k, ctx_h, e_aps)
        if k == NK - 1:
            emit_ctx(*pend_ctx)       # flush before the block-end chunk
            pend_ctx = None
        # previous q-block's out-projection, spread over this block's
        # PE/DVE slack: matmuls (vs unnormalized ctx -- no wait on the
        # normalize chain) every 3rd iteration, scale-combines after.
        # Spacing matters: bursting these backs up the DVE with o_raw
        # copies and the in-order PE queue stalls behind an out_mm
        # waiting on it, starving the exp stream.
        if prev is not None:
            if k in mm_pos:
                out_mm(prev, mm_pos[k])
            if k in ml_pos:
                out_mul(prev, ml_pos[k])
            if k in ad_pos:
                out_add(prev, ad_pos[k])
        # next q-block's Q projection, 4 iterations before its consumer
        # (the score-ahead emission at k == NK-1)
        if k == NK - 8 and qb + 1 < NQB:
            proj_block(qt_sb, "wq", (qb + 1) * 512)
        if k == 2 and prev is not None and prev["bc"] is None:
            prev["bc"] = normalize_b(prev["r64"])
        if k == NK - 1:
            last = (qb == NQB - 1)
            # normalize phase A first: its tiny r2 copies unblock the DMA
            # bounce before the boundary staging work
            r64 = normalize_a(ctx_blocks[qb])
            if not last:
                # stage the unnormalized context out of PSUM (bf16); the
                # 1/rowsum scales are applied by out_mul later
                for h in range(2):
                    nc.vector.tensor_copy(ctx_sb[h * DH:(h + 1) * DH, qsl],
                                          ctx_h[h][:DH, :])
            prev = {"qsl": qsl, "raw": [None] * NC_T, "mul": [None] * NC_T,
                    "bc": None, "r64": r64}
    # final q-block: nothing left to pipeline against, so take the short
    # path -- fused scale+cast of the context out of PSUM once bc lands
    # (the DVE is otherwise idle, so the normalize chain runs unclogged),
    # then full-K out-projections
    if prev["bc"] is None:
        prev["bc"] = normalize_b(prev["r64"])
    qsl = prev["qsl"]
    for h in range(2):
        nc.vector.tensor_mul(ctx_sb[h * DH:(h + 1) * DH, qsl],
                             ctx_blocks[NQB - 1][h][:DH, :],
                             prev["bc"][h][0:DH, :])
    for idx in range(NC_T):
        o_ps = ps.tile([P, QB], F32, tag="pp", bufs=2, name="o_ps")
        mm(o_ps[:], wo_sb[:, idx * P:(idx + 1) * P], ctx_sb[:, qsl])
        o_sb = osb.tile([P, QB], F32, tag="osb", bufs=2, name="o_sb")
        nc.vector.tensor_copy(o_sb[:], o_ps[:])
        nc.sync.dma_start(out=yt[idx * P:(idx + 1) * P, qsl], in_=o_sb[:])


_TPB_ENGINES = {mybir.EngineType.PE, mybir.EngineType.Activation,
                mybir.EngineType.DVE, mybir.EngineType.Pool}


def _legalize_matmul_waits(nc: bass.Bass) -> int:
    """Walrus encodes only ONE sync wait on TPB compute instructions (seen on
    Matmult and TensorCopy).  Move extra waits onto injected same-engine
    no-ops (one wait each) placed immediately before the instruction in its
    block: same semantics, legal encoding."""
    n_fixed = 0
    for f in nc.m.functions:
        for bb in f.blocks:
            out = []
            changed = False
            for ins in bb.instructions:
                si = ins.sync_info
                if (getattr(ins, "engine", None) is not None
                        and si is not None and len(si.on_wait) > 1):
                    for idx, w in enumerate(si.on_wait[:-1]):
                        nop = mybir.InstNoOp(name=f"{ins.name}-lgw{idx}",
                                             ins=[], outs=[])
                        nop.engine = ins.engine
                        nop.sync_info = mybir.SyncInfo(on_wait=[w], on_update=[])
                        out.append(nop)
                    ins.sync_info = mybir.SyncInfo(on_wait=[si.on_wait[-1]],
                                                   on_update=si.on_update)
                    n_fixed += 1
                    changed = True
                out.append(ins)
            if changed:
                bb.instructions = out
    return n_fixed


def build(S: int = S_FULL, legalize: bool = False) -> bass.Bass:
    nc = bass.Bass()
    with ExitStack() as ctx:
        ctx.enter_context(nc.allow_low_precision(
            reason="bf16 matmul operands / intermediates"))
        tc = ctx.enter_context(tile.TileContext(nc))
        _emit(nc, tc, ctx, S)
    if legalize:
        # only for the walrus/hardware path; CoreSim wants updates on every
        # instruction and doesn't enforce the 1-wait Matmult limit
        _legalize_matmul_waits(nc)
    return nc


_NC_CACHE = {}


def _get_nc(S: int) -> bass.Bass:
    if S not in _NC_CACHE:
        _NC_CACHE[S] = build(S, legalize=True)
    return _NC_CACHE[S]


def _bf16(a):
    import ml_dtypes
    return np.ascontiguousarray(np.asarray(a, dtype=np.float32)).astype(
        ml_dtypes.bfloat16)


def make_in_maps(X, Wq, Wk, Wv, Wo):
    X = np.asarray(X, dtype=np.float32)
    xts = [_bf16(X[b].T) for b in range(B)]
    in_maps = []
    for i in range(N_CORES):
        b, hp = divmod(i, 4)  # 4 head-pairs per batch
        csl = slice(hp * P, (hp + 1) * P)
        in_maps.append({
            "xt": xts[b],
            "wq": _bf16(Wq[:, csl]),
            "wk": _bf16(Wk[:, csl]),
            "wv": _bf16(Wv[:, csl]),
            "wo": _bf16(Wo[csl, :]),
        })
    return in_maps


def kernel(X, Wq, Wk, Wv, Wo, _trace=False):
    global LAST_RESULTS
    X = np.asarray(X, dtype=np.float32)
    S = X.shape[1]
    nc = _get_nc(S)
    in_maps = make_in_maps(X, np.asarray(Wq, np.float32), np.asarray(Wk, np.float32),
                           np.asarray(Wv, np.float32), np.asarray(Wo, np.float32))
    res = run_bass_kernel_spmd(nc, in_maps, list(range(N_CORES)), trace=_trace)
    LAST_RESULTS = res
    Y = np.zeros((B, S, D), dtype=np.float32)
    for i in range(N_CORES):
        Y[i // 4] += res.results[i]["yt"].T
    return Y

